# revision 46
# baseline (speedup 1.0000x reference)
"""Trainium2 Bass kernel for nn_Attention_34351148434119 (8 NeuronCores).

Reference computation (faithful quirks included):
  q_proj = hid @ Wq; q, gate = split(q_proj)     # q is DEAD code downstream
  k = hid @ Wk; v = hid @ Wv                     # [B,KV,S,D]
  v = RoPE(v)  (k is NOT roped; q roped but unused)
  scores = (k @ v^T) * sqrt(D) + mask; attn = softmax_t(scores)   # per kv head
  out = (tile_G(attn @ v) * sigmoid(gate)) @ Wo

Sharding: core = b*4 + j  (b = batch, j = rank in 4-core batch group).
Per batch, S=2048 is split into 16 blocks of 128 rows; core j owns blocks
{4k+j} (slot k block = 4k+j) so every core has an identical causal
workload (uniform SPMD graph; per-core specialization only via staged
data).  Row-major v (fp16) is shared within each batch group by ONE
AllGather; the d-major score-side tiles are rebuilt post-AG with 64
cheap PE transposes (keeps all post-AG DMA loads 1KB-row contiguous).

Ordering: the v projection runs first so the AllGather fires ~25us in;
the gate matmul is emitted last (lowest priority) so its dense PE work
fills the AG wait and every softmax bubble.

Precision: the whole k/v chain runs 1-pass fp16 (m11 rounding at every
matmul input; PSUM accumulation fp32); host-side simulation of this
rounding chain measures rel err 5.1e-3 vs the 2e-2 gate.  Softmax is
two-phase; the 1/sum normalization rides the [s,t]->[t,s] transpose
for free: the transpose is a plain fp16 matmul against diag(1/sum)
instead of the identity, and the 4 sub-transposes of each (slot,chunk)
land in one PSUM bank that is evacuated with a single strided copy.
All HBM staging is fp16/bf16 with host-packed layouts so every big DMA
is >=2KB contiguous per partition row.
"""
import sys
import numpy as np

sys.path.insert(0, "/opt/trn_rl_repo")

B, S, HS = 2, 2048, 2048
H, KV, D = 16, 4, 128
G = H // KV
SCALING = float(D) ** 0.5
P = 128
NB = S // P            # 16 row blocks per batch
NCORES = 8
RANKS = 4              # cores per batch group
SLOTS = 4              # owned 128-row blocks per core
ROWS = SLOTS * P       # 512 rows per core
CHUNK = 512            # t-chunk = 4 t-tiles
NCHUNK = S // CHUNK    # 4
KT = HS // P           # 16 contraction tiles
KVD = KV * D
NEG_THRESH = -1e8

_CACHE = {}


def _mask_classes(mask):
    """Classify each (s-slot k, t-chunk c) 512x512 region of the SxS mask.

    0 = skip (everything <= NEG_THRESH: contributes exact 0 after softmax)
    1 = plain (all zeros: no add needed)
    2 = add  (mixed: stage values and add on-chip)
    Slot k rows across all cores = blocks 4k..4k+3 = rows [512k, 512k+512).
    """
    cls = [[0] * NCHUNK for _ in range(SLOTS)]
    for k in range(SLOTS):
        for c in range(NCHUNK):
            reg = mask[512 * k:512 * (k + 1), 512 * c:512 * (c + 1)]
            if (reg <= NEG_THRESH).all():
                cls[k][c] = 0
            elif (reg == 0).all():
                cls[k][c] = 1
            else:
                cls[k][c] = 2
    ok = True
    for k in range(SLOTS):
        comp = [c for c in range(NCHUNK) if cls[k][c] != 0]
        # computed chunks must be a prefix starting at 0
        if comp != list(range(len(comp))) or 0 not in comp:
            ok = False
    if ok:
        # {k : chunk c computed} must be a suffix of slots for each c
        for c in range(NCHUNK):
            ks = [k for k in range(SLOTS) if cls[k][c] != 0]
            if ks != list(range(SLOTS - len(ks), SLOTS)):
                ok = False
    if not ok:
        # fully dense fallback: always correct for any mask
        cls = [[2] * NCHUNK for _ in range(SLOTS)]
    return cls


def _build(classes):
    from contextlib import ExitStack

    from concourse import bacc, mybir, tile
    from concourse.masks import make_identity

    f32 = mybir.dt.float32
    f16 = mybir.dt.float16
    bf16 = mybir.dt.bfloat16
    f8 = mybir.dt.float8e4
    Alu = mybir.AluOpType
    Act = mybir.ActivationFunctionType
    DR = mybir.MatmulPerfMode.DoubleRow

    computed = [[c for c in range(NCHUNK) if classes[k][c] != 0] for k in range(SLOTS)]
    add_idx = {}
    for k in range(SLOTS):
        for c in range(NCHUNK):
            if classes[k][c] == 2:
                add_idx[(k, c)] = len(add_idx)
    n_add = max(len(add_idx), 1)

    nc = bacc.Bacc("TRN2", target_bir_lowering=False, debug=False,
                   num_devices=NCORES)

    # host-packed layouts: partition row p carries kk-tiles side by side so
    # every DMA moves long contiguous runs per partition.
    hid_d = nc.declare_dram_parameter("hid16", [P, KT * CHUNK], f16, isOutput=False)
    wv_d = nc.declare_dram_parameter("wv16", [P, KT * KVD], f16, isOutput=False)
    wk_d = nc.declare_dram_parameter("wk16", [P, KT * KVD], f16, isOutput=False)
    wqg_d = nc.declare_dram_parameter("wqg8", [16, P, KT, P], f8,
                                      isOutput=False)
    hid8_d = nc.declare_dram_parameter("hid8", [P, KT, CHUNK], f8,
                                       isOutput=False)
    wo_d = nc.declare_dram_parameter("wo16", [4, P, KT * CHUNK], f16,
                                     isOutput=False)
    cosT_d = nc.declare_dram_parameter("cosT", [D, ROWS], f16, isOutput=False)
    sinT_d = nc.declare_dram_parameter("sinT", [D, ROWS], f16, isOutput=False)
    mask_d = nc.declare_dram_parameter("maskst", [P, n_add * CHUNK], bf16,
                                       isOutput=False)
    out_d = nc.declare_dram_parameter("out", [ROWS, HS], bf16, isOutput=True)

    rg = [[0, 1, 2, 3], [4, 5, 6, 7]]

    with tile.TileContext(nc) as tc, ExitStack() as ctx:
        sb = ctx.enter_context(tc.tile_pool(name="sb", bufs=2))
        ps = ctx.enter_context(tc.tile_pool(name="ps", bufs=8, space="PSUM"))
        dram = ctx.enter_context(tc.tile_pool(name="dram", bufs=1, space="DRAM"))

        # ---- constants ----
        id16 = sb.tile([P, P], f16, tag="c_id16", bufs=1)
        make_identity(nc, id16[:])

        # ---- input DMAs (chunked; first proj chunk leads the queue) ----
        hidt = sb.tile([P, KT * CHUNK], f16, tag="hid", bufs=1)
        w = KT * CHUNK // 4
        nc.sync.dma_start(hidt[:, 0:w], hid_d[:, 0:w])
        wvts = []
        for q in range(4):
            wvt = sb.tile([P, 4 * KVD], f16, tag="wv", bufs=2, name=f"wv{q}")
            wvts.append(wvt)
        nc.sync.dma_start(wvts[0][:], wv_d[:, 0:4 * KVD])
        cosT = sb.tile([D, ROWS], f16, tag="c_cos", bufs=1)
        sinT = sb.tile([D, ROWS], f16, tag="c_sin", bufs=1)
        nc.sync.dma_start(cosT[:], cosT_d[:, :])
        nc.sync.dma_start(sinT[:], sinT_d[:, :])
        for q in range(1, 4):
            nc.sync.dma_start(hidt[:, q * w:(q + 1) * w],
                              hid_d[:, q * w:(q + 1) * w])
        nc.sync.dma_start(wvts[1][:], wv_d[:, 4 * KVD:8 * KVD])

        # ---- v projection first (1-pass fp16) so the AG can fire early ----
        pv = [ps.tile([P, ROWS], f32, tag="ps", name=f"pv{g}") for g in range(KV)]
        for q in range(4):
            wvt = wvts[q]
            for kq in range(4):
                kk = q * 4 + kq
                rhs = hidt[:, kk * CHUNK:(kk + 1) * CHUNK]
                for g in range(KV):
                    nc.tensor.matmul(pv[g][:],
                                     wvt[:, kq * KVD + g * P:kq * KVD + (g + 1) * P],
                                     rhs, start=(kk == 0), stop=(kk == KT - 1))
            if q + 2 < 4:
                # emit late so a busy slot never blocks the DMA FIFO
                nc.sync.dma_start(wvts[q + 2][:],
                                  wv_d[:, (q + 2) * 4 * KVD:(q + 3) * 4 * KVD])

        # RoPE on v (f32 DVE) then round to fp16; transpose to row-major
        vrow = [sb.tile([P, KVD], f16, tag="vrow", bufs=4, name=f"vrow{rt}")
                for rt in range(SLOTS)]
        for g in range(KV):
            vr = sb.tile([P, ROWS], f32, tag="vr", bufs=1)
            nc.scalar.copy(vr[:], pv[g][:])
            rot = sb.tile([P, ROWS], f32, tag="rot", bufs=1)
            nc.vector.tensor_scalar_mul(rot[0:64, :], vr[64:128, :], -1.0)
            nc.vector.tensor_copy(rot[64:128, :], vr[0:64, :])
            nc.vector.tensor_mul(vr[:], vr[:], cosT[:])
            nc.vector.tensor_mul(rot[:], rot[:], sinT[:])
            vr16 = sb.tile([P, ROWS], f16, tag="vr16", bufs=1)
            nc.vector.tensor_add(vr16[:], vr[:], rot[:])
            for rt in range(SLOTS):
                tp = ps.tile([P, P], f16, tag="ps", name=f"tpv{g}_{rt}")
                nc.tensor.transpose(tp[:], vr16[:, rt * P:(rt + 1) * P], id16[:])
                if rt % 2:
                    nc.scalar.copy(vrow[rt][:, g * P:(g + 1) * P], tp[:])
                else:
                    nc.vector.tensor_copy(vrow[rt][:, g * P:(g + 1) * P], tp[:])

        # ---- ONE AllGather of row-major v (within 4-core batch group) ----
        vr_in = dram.tile([ROWS, KVD], f16)
        vr_all = dram.tile([RANKS * ROWS, KVD], f16)
        for rt in range(SLOTS):
            nc.gpsimd.dma_start(vr_in[rt * P:(rt + 1) * P, :], vrow[rt][:])
        nc.gpsimd.collective_compute(
            "AllGather", mybir.AluOpType.bypass, replica_groups=rg,
            ins=[vr_in.opt()], outs=[vr_all.opt()])

        # ---- k projection (1-pass fp16) while the AG runs ----
        pk = [ps.tile([P, ROWS], f32, tag="ps", name=f"pk{g}") for g in range(KV)]
        for q in range(4):
            wkt = sb.tile([P, 4 * KVD], f16, tag="wk", bufs=2, name=f"wk{q}")
            nc.scalar.dma_start(wkt[:], wk_d[:, q * 4 * KVD:(q + 1) * 4 * KVD])
            for kq in range(4):
                kk = q * 4 + kq
                rhs = hidt[:, kk * CHUNK:(kk + 1) * CHUNK]
                for g in range(KV):
                    nc.tensor.matmul(pk[g][:],
                                     wkt[:, kq * KVD + g * P:kq * KVD + (g + 1) * P],
                                     rhs, start=(kk == 0), stop=(kk == KT - 1))
        kT16 = []
        for g in range(KV):
            t = sb.tile([P, ROWS], f16, tag="kT", bufs=4, name=f"kT{g}")
            nc.scalar.mul(t[:], pk[g][:], SCALING)
            kT16.append(t)

        # ---- gate slab helper (fp8 DoubleRow matmul; gating deferred) ----
        # explicit interleaving: engine streams are static, so gate slabs are
        # emitted exactly where their dense PE work fills a stall (the AG
        # wait, each head's softmax bubbles).  The sigmoid is computed later
        # as exp(-x) (shares the ACT Exp table with the softmax - a
        # Sigmoid<->Exp switch reloads the table at 1.3us per switch) plus
        # cheap DVE ops fused into the gating multiply.
        sigT = [None] * 16
        hid8t = sb.tile([P, KT, CHUNK], f8, tag="hid8", bufs=1)
        nc.scalar.dma_start(hid8t[:], hid8_d[:, :, :])

        def gate_slab(nb):
            wq = sb.tile([P, KT, P], f8, tag="wqs", bufs=2, name=f"wq{nb}")
            nc.sync.dma_start(wq[:], wqg_d[nb, :, :, :])
            pg = ps.tile([P, ROWS], f32, tag="ps", name=f"pg{nb}")
            for t2 in range(KT // 2):
                nc.tensor.matmul(pg[:], wq[:, 2 * t2:2 * t2 + 2, :],
                                 hid8t[:, 2 * t2:2 * t2 + 2, :],
                                 start=(t2 == 0), stop=(t2 == KT // 2 - 1),
                                 perf_mode=DR)
            t = sb.tile([P, ROWS], f16, tag="sigT", bufs=16, name=f"sig{nb}")
            nc.vector.tensor_copy(t[:], pg[:])
            sigT[nb] = t

        for nb in range(9):
            gate_slab(nb)

        # ---- gathered loads (contiguous [128, 512] tiles) ----
        # vr_all row-block i = r*SLOTS + rt holds global t-block bi = 4*rt + r.
        vrg = []
        for i in range(RANKS * SLOTS):
            t = sb.tile([P, KVD], f16, tag="vrg", bufs=16, name=f"vrg{i}")
            if i % 2:
                nc.scalar.dma_start(t[:], vr_all[i * P:(i + 1) * P, :])
            else:
                nc.gpsimd.dma_start(t[:], vr_all[i * P:(i + 1) * P, :])
            vrg.append(t)

        # rebuild d-major score-side tiles with PE transposes
        # vtc[g*NCHUNK+c]: [128 d(g), 512 t], t-block 4c+r from vrg[r*SLOTS+c]
        vtc = [sb.tile([P, CHUNK], f16, tag="vtc", bufs=16, name=f"vtc{i}")
               for i in range(KV * NCHUNK)]
        for c in range(NCHUNK):
            for r in range(RANKS):
                src = vrg[r * SLOTS + c]
                for g in range(KV):
                    tp = ps.tile([P, P], f16, tag="ps", name=f"tpc{c}_{r}_{g}")
                    nc.tensor.transpose(tp[:], src[:, g * P:(g + 1) * P], id16[:])
                    dst = vtc[g * NCHUNK + c][:, r * P:(r + 1) * P]
                    nc.vector.tensor_copy(dst, tp[:])

        wos = [None] * 4

        # ---- attention per kv head, gate slabs filling the bubbles ----
        avT = [None] * KV
        gat = [None] * (KV * G)
        mts = {}
        for g in range(KV):
            # attnT_big[t(128), bi(16), s(512)]: normalized probs, [t,s] layout
            attnTB = sb.tile([P, NB, ROWS], f16, tag="attnTB", bufs=2,
                             name=f"attnTB{g}")

            def emit_tpa(k, comp, attn, dg):
                # transpose+normalize attn -> attnTB columns (bi, slot k);
                # deferred one slot so the PE stream never head-of-line
                # blocks on this slot's softmax chain.
                for ci, c in enumerate(comp):
                    tp = ps.tile([P, 4, P], f32, tag="ps", name=f"tpa{k}_{ci}")
                    for i in range(4):
                        nc.tensor.matmul(
                            tp[:, i, :],
                            attn[:, ci * CHUNK + i * P:ci * CHUNK + (i + 1) * P],
                            dg[:], start=True, stop=True)
                    nc.vector.tensor_copy(
                        attnTB[:, 4 * c:4 * c + 4, k * P:(k + 1) * P], tp[:])

            pending = None
            for k in range(SLOTS):
                comp = computed[k]
                nchk = len(comp)
                attn = sb.tile([P, CHUNK * nchk], f16, tag="attn", bufs=2,
                               padded_shape=[P, CHUNK * NCHUNK],
                               name=f"attn{g}_{k}")
                # two-phase softmax: all chunk matmuls + maxes run in
                # parallel, then one exp wave with the final max.
                pscs = []
                cms = []
                for ci, c in enumerate(comp):
                    psc = ps.tile([P, CHUNK], f32, tag="ps", name=f"psc{ci}")
                    nc.tensor.matmul(psc[:], kT16[g][:, k * P:(k + 1) * P],
                                     vtc[g * NCHUNK + c][:], start=True, stop=True)
                    if classes[k][c] == 2:
                        ai = add_idx[(k, c)]
                        if ai not in mts:
                            mt = sb.tile([P, CHUNK], bf16, tag="msk", bufs=4,
                                         name=f"mt{ai}")
                            nc.gpsimd.dma_start(
                                mt[:], mask_d[:, ai * CHUNK:(ai + 1) * CHUNK])
                            mts[ai] = mt
                        nc.vector.tensor_add(psc[:], psc[:], mts[ai][:])
                    cm = sb.tile([P, 1], f32, tag="stat", bufs=16, name=f"cm{ci}")
                    nc.vector.tensor_reduce(cm[:], psc[:], mybir.AxisListType.X,
                                            Alu.max, negate=True)
                    pscs.append(psc)
                    cms.append(cm)
                mneg = cms[0]   # -max
                for ci in range(1, nchk):
                    mnew = sb.tile([P, 1], f32, tag="stat", bufs=16, name=f"mn{ci}")
                    nc.vector.tensor_tensor(mnew[:], mneg[:], cms[ci][:], Alu.min)
                    mneg = mnew
                tot = None
                for ci in range(nchk):
                    csum = sb.tile([P, 1], f32, tag="stat", bufs=16, name=f"cs{ci}")
                    nc.scalar.activation(attn[:, ci * CHUNK:(ci + 1) * CHUNK],
                                         pscs[ci][:], Act.Exp, bias=mneg[:],
                                         accum_out=csum[:])
                    if tot is None:
                        tot = csum
                    else:
                        t2 = sb.tile([P, 1], f32, tag="stat", bufs=16, name=f"tt{ci}")
                        nc.vector.tensor_add(t2[:], tot[:], csum[:])
                        tot = t2
                rinv = sb.tile([P, 1], f32, tag="stat", bufs=16)
                nc.vector.reciprocal(rinv[:], tot[:])
                # diag(1/sum): the transpose matmul applies the softmax
                # normalization for free.
                dg = sb.tile([P, P], f16, tag="dg", bufs=2, name=f"dg{g}_{k}")
                nc.vector.tensor_scalar_mul(dg[:], id16[:], rinv[:])
                if k % 2 and 9 + 2 * g + k // 2 < 16:
                    gate_slab(9 + 2 * g + k // 2)
                if pending is not None:
                    emit_tpa(*pending)
                pending = (k, comp, attn, dg)
            emit_tpa(*pending)
            # attn @ v  ->  avT[g] [128 d, 512 s]
            todo = []
            for bi in range(NB):
                ks = [k for k in range(SLOTS) if (bi // RANKS) in computed[k]]
                if ks:
                    todo.append((bi, ks[0]))
            pav = ps.tile([P, ROWS], f32, tag="ps")
            for n, (bi, kmin) in enumerate(todo):
                lhs = vrg[(bi % RANKS) * SLOTS + (bi // RANKS)][:, g * P:(g + 1) * P]
                nc.tensor.matmul(pav[:, kmin * P:ROWS], lhs,
                                 attnTB[:, bi, kmin * P:ROWS],
                                 start=(n == 0), stop=(n == len(todo) - 1))
            t = sb.tile([P, ROWS], f16, tag="avT", bufs=4, name=f"avT{g}")
            nc.vector.tensor_copy(t[:], pav[:])
            avT[g] = t
            if g == 1:
                wos[0] = sb.tile([P, KT * CHUNK], f16, tag="wos", bufs=2,
                                 name="wo0")
                nc.sync.dma_start(wos[0][:], wo_d[0, :, :])
        # ---- gated = tile_G(avT) * sigmoid(glog), in place in sigT ----
        # sigmoid(x) = 1 / (1 + exp(-x)): the Exp shares the softmax's ACT
        # table (no reload); add/reciprocal/mul run on DVE.
        with nc.allow_low_precision(reason="sigmoid on fp16 gate logits"):
            for g in range(KV):
                for i in range(G):
                    nb = 4 * g + i
                    nc.scalar.activation(sigT[nb][:], sigT[nb][:], Act.Exp,
                                         scale=-1.0)
                    nc.vector.tensor_scalar_add(sigT[nb][:], sigT[nb][:], 1.0)
                    nc.vector.reciprocal(sigT[nb][:], sigT[nb][:])
                    nc.vector.tensor_mul(sigT[nb][:], avT[g][:], sigT[nb][:])
                    gat[nb] = sigT[nb]

        # ---- out projection (fp16) ----
        for nblk in range(4):
            if wos[nblk] is None:
                wos[nblk] = sb.tile([P, KT * CHUNK], f16, tag="wos", bufs=2,
                                    name=f"wo{nblk}")
                nc.sync.dma_start(wos[nblk][:], wo_d[nblk, :, :])
            wob = wos[nblk]
            for rt in range(SLOTS):
                po = ps.tile([P, CHUNK], f32, tag="ps")
                for cc in range(KT):
                    nc.tensor.matmul(po[:], gat[cc][:, rt * P:(rt + 1) * P],
                                     wob[:, cc * CHUNK:(cc + 1) * CHUNK],
                                     start=(cc == 0), stop=(cc == KT - 1))
                t = sb.tile([P, CHUNK], bf16, tag="oev", bufs=2,
                            name=f"oev{nblk}_{rt}")
                if rt % 2:
                    nc.vector.tensor_copy(t[:], po[:])
                else:
                    nc.scalar.copy(t[:], po[:])
                nc.sync.dma_start(
                    out_d[rt * P:(rt + 1) * P, nblk * CHUNK:(nblk + 1) * CHUNK], t[:])
            if nblk + 1 < 4 and wos[nblk + 1] is None:
                wos[nblk + 1] = sb.tile([P, KT * CHUNK], f16, tag="wos", bufs=2,
                                        name=f"wo{nblk + 1}")
                nc.sync.dma_start(wos[nblk + 1][:], wo_d[nblk + 1, :, :])

    nc.compile()
    return nc


def _pack_kt(w):
    """[HS, N] -> [128, KT*N] with row p carrying all kk-tiles contiguously."""
    n = w.shape[1]
    return np.ascontiguousarray(
        w.reshape(KT, P, n).transpose(1, 0, 2).reshape(P, KT * n))


def kernel(hidden_states, cos, sin, attention_mask, Wq, Wk, Wv, Wo):
    import ml_dtypes
    from concourse.bass_utils import run_bass_kernel_spmd

    hidden_states = np.asarray(hidden_states, dtype=np.float32)
    cos = np.asarray(cos, dtype=np.float32)
    sin = np.asarray(sin, dtype=np.float32)
    mask = np.asarray(attention_mask, dtype=np.float32)[0, 0]
    Wq = np.asarray(Wq, dtype=np.float32)
    Wk = np.asarray(Wk, dtype=np.float32)
    Wv = np.asarray(Wv, dtype=np.float32)
    Wo = np.asarray(Wo, dtype=np.float32)

    classes = _mask_classes(mask)
    key = tuple(tuple(r) for r in classes)
    if key not in _CACHE:
        _CACHE[key] = _build(classes)
    nc = _CACHE[key]

    # shared (identical across cores) staged weights, packed + fp16
    wv16 = _pack_kt(Wv).astype(np.float16)
    wk16 = _pack_kt(Wk).astype(np.float16)
    wqg8 = np.ascontiguousarray(
        Wq[:, HS:].reshape(KT, P, 16, P).transpose(2, 1, 0, 3)).astype(
            ml_dtypes.float8_e4m3fn)
    wo16 = np.ascontiguousarray(
        Wo.reshape(KT, P, 4, CHUNK).transpose(2, 1, 0, 3)
        .reshape(4, P, KT * CHUNK)).astype(np.float16)

    in_maps = []
    for core in range(NCORES):
        b, j = divmod(core, RANKS)
        blocks = [RANKS * k + j for k in range(SLOTS)]
        rows = np.concatenate([np.arange(bi * P, (bi + 1) * P) for bi in blocks])
        strips = []
        for k in range(SLOTS):
            for c in range(NCHUNK):
                if classes[k][c] == 2:
                    bi = RANKS * k + j
                    strips.append(mask[bi * P:(bi + 1) * P,
                                       c * CHUNK:(c + 1) * CHUNK])
        if not strips:
            strips.append(np.zeros((P, CHUNK), np.float32))
        maskst = np.concatenate(strips, axis=1).astype(ml_dtypes.bfloat16)
        hidT3 = np.ascontiguousarray(
            hidden_states[b][rows].T).reshape(KT, P, CHUNK).transpose(1, 0, 2)
        hid16 = np.ascontiguousarray(
            hidT3.reshape(P, KT * CHUNK)).astype(np.float16)
        hid8 = np.ascontiguousarray(hidT3).astype(ml_dtypes.float8_e4m3fn)
        in_maps.append({
            "hid16": hid16,
            "hid8": hid8,
            "wv16": wv16,
            "wk16": wk16,
            "wqg8": wqg8,
            "wo16": wo16,
            "cosT": np.ascontiguousarray(cos[b][rows].T).astype(np.float16),
            "sinT": np.ascontiguousarray(sin[b][rows].T).astype(np.float16),
            "maskst": np.ascontiguousarray(maskst),
        })

    res = run_bass_kernel_spmd(nc, in_maps, core_ids=list(range(NCORES)))

    out = np.empty((B, S, HS), np.float32)
    for core in range(NCORES):
        b, j = divmod(core, RANKS)
        o = np.asarray(res.results[core]["out"]).astype(np.float32)
        for k in range(SLOTS):
            bi = RANKS * k + j
            out[b, bi * P:(bi + 1) * P, :] = o[k * P:(k + 1) * P, :]
    return out


# revision 48
# speedup vs baseline: 1.0698x; 1.0698x over previous
"""Trainium2 Bass kernel for nn_Attention_34351148434119 (8 NeuronCores).

Reference computation (faithful quirks included):
  q_proj = hid @ Wq; q, gate = split(q_proj)     # q is DEAD code downstream
  k = hid @ Wk; v = hid @ Wv                     # [B,KV,S,D]
  v = RoPE(v)  (k is NOT roped; q roped but unused)
  scores = (k @ v^T) * sqrt(D) + mask; attn = softmax_t(scores)   # per kv head
  out = (tile_G(attn @ v) * sigmoid(gate)) @ Wo

Sharding: core = b*4 + j  (b = batch, j = rank in 4-core batch group).
Per batch, S=2048 is split into 16 blocks of 128 rows; core j owns blocks
{4k+j} (slot k block = 4k+j) so every core has an identical causal
workload (uniform SPMD graph; per-core specialization only via staged
data).  Row-major v (fp16) is shared within each batch group by ONE
AllGather; the d-major score-side tiles are rebuilt post-AG with 64
cheap PE transposes (keeps all post-AG DMA loads 1KB-row contiguous).

Ordering: the v projection runs first so the AllGather fires ~25us in;
the gate matmul is emitted last (lowest priority) so its dense PE work
fills the AG wait and every softmax bubble.

Precision: the whole k/v chain runs 1-pass fp16 (m11 rounding at every
matmul input; PSUM accumulation fp32); host-side simulation of this
rounding chain measures rel err 5.1e-3 vs the 2e-2 gate.  Softmax is
two-phase; the 1/sum normalization rides the [s,t]->[t,s] transpose
for free: the transpose is a plain fp16 matmul against diag(1/sum)
instead of the identity, and the 4 sub-transposes of each (slot,chunk)
land in one PSUM bank that is evacuated with a single strided copy.
All HBM staging is fp16/bf16 with host-packed layouts so every big DMA
is >=2KB contiguous per partition row.
"""
import sys
import numpy as np

sys.path.insert(0, "/opt/trn_rl_repo")

B, S, HS = 2, 2048, 2048
H, KV, D = 16, 4, 128
G = H // KV
SCALING = float(D) ** 0.5
P = 128
NB = S // P            # 16 row blocks per batch
NCORES = 8
RANKS = 4              # cores per batch group
SLOTS = 4              # owned 128-row blocks per core
ROWS = SLOTS * P       # 512 rows per core
CHUNK = 512            # t-chunk = 4 t-tiles
NCHUNK = S // CHUNK    # 4
KT = HS // P           # 16 contraction tiles
KVD = KV * D
NEG_THRESH = -1e8

_CACHE = {}


def _mask_classes(mask):
    """Classify each (s-slot k, t-chunk c) 512x512 region of the SxS mask.

    0 = skip (everything <= NEG_THRESH: contributes exact 0 after softmax)
    1 = plain (all zeros: no add needed)
    2 = add  (mixed: stage values and add on-chip)
    Slot k rows across all cores = blocks 4k..4k+3 = rows [512k, 512k+512).
    """
    cls = [[0] * NCHUNK for _ in range(SLOTS)]
    for k in range(SLOTS):
        for c in range(NCHUNK):
            reg = mask[512 * k:512 * (k + 1), 512 * c:512 * (c + 1)]
            if (reg <= NEG_THRESH).all():
                cls[k][c] = 0
            elif (reg == 0).all():
                cls[k][c] = 1
            else:
                cls[k][c] = 2
    ok = True
    for k in range(SLOTS):
        comp = [c for c in range(NCHUNK) if cls[k][c] != 0]
        # computed chunks must be a prefix starting at 0
        if comp != list(range(len(comp))) or 0 not in comp:
            ok = False
    if ok:
        # {k : chunk c computed} must be a suffix of slots for each c
        for c in range(NCHUNK):
            ks = [k for k in range(SLOTS) if cls[k][c] != 0]
            if ks != list(range(SLOTS - len(ks), SLOTS)):
                ok = False
    if not ok:
        # fully dense fallback: always correct for any mask
        cls = [[2] * NCHUNK for _ in range(SLOTS)]
    return cls


def _build(classes):
    from contextlib import ExitStack

    from concourse import bacc, mybir, tile
    from concourse.masks import make_identity

    f32 = mybir.dt.float32
    f16 = mybir.dt.float16
    bf16 = mybir.dt.bfloat16
    Alu = mybir.AluOpType
    Act = mybir.ActivationFunctionType

    computed = [[c for c in range(NCHUNK) if classes[k][c] != 0] for k in range(SLOTS)]
    add_idx = {}
    for k in range(SLOTS):
        for c in range(NCHUNK):
            if classes[k][c] == 2:
                add_idx[(k, c)] = len(add_idx)
    n_add = max(len(add_idx), 1)

    nc = bacc.Bacc("TRN2", target_bir_lowering=False, debug=False,
                   num_devices=NCORES)

    # host-packed layouts: partition row p carries kk-tiles side by side so
    # every DMA moves long contiguous runs per partition.
    hid_d = nc.declare_dram_parameter("hid16", [P, KT * CHUNK], f16, isOutput=False)
    wv_d = nc.declare_dram_parameter("wv16", [P, KT * KVD], f16, isOutput=False)
    wk_d = nc.declare_dram_parameter("wk16", [P, KT * KVD], f16, isOutput=False)
    wqg_d = nc.declare_dram_parameter("wqg16", [16, P, KT * P], f16,
                                      isOutput=False)
    wo_d = nc.declare_dram_parameter("wo16", [4, P, KT * CHUNK], f16,
                                     isOutput=False)
    cosT_d = nc.declare_dram_parameter("cosT", [D, ROWS], f16, isOutput=False)
    sinT_d = nc.declare_dram_parameter("sinT", [D, ROWS], f16, isOutput=False)
    mask_d = nc.declare_dram_parameter("maskst", [P, n_add * CHUNK], bf16,
                                       isOutput=False)
    out_d = nc.declare_dram_parameter("out", [ROWS, HS], bf16, isOutput=True)

    rg = [[0, 1, 2, 3], [4, 5, 6, 7]]

    with tile.TileContext(nc) as tc, ExitStack() as ctx:
        sb = ctx.enter_context(tc.tile_pool(name="sb", bufs=2))
        ps = ctx.enter_context(tc.tile_pool(name="ps", bufs=8, space="PSUM"))
        dram = ctx.enter_context(tc.tile_pool(name="dram", bufs=1, space="DRAM"))

        # ---- constants ----
        id16 = sb.tile([P, P], f16, tag="c_id16", bufs=1)
        make_identity(nc, id16[:])

        # ---- input DMAs (chunked; first proj chunk leads the queue) ----
        hidt = sb.tile([P, KT * CHUNK], f16, tag="hid", bufs=1)
        w = KT * CHUNK // 4
        nc.sync.dma_start(hidt[:, 0:w], hid_d[:, 0:w])
        wvts = []
        for q in range(4):
            wvt = sb.tile([P, 4 * KVD], f16, tag="wv", bufs=2, name=f"wv{q}")
            wvts.append(wvt)
        nc.sync.dma_start(wvts[0][:], wv_d[:, 0:4 * KVD])
        cosT = sb.tile([D, ROWS], f16, tag="c_cos", bufs=1)
        sinT = sb.tile([D, ROWS], f16, tag="c_sin", bufs=1)
        nc.sync.dma_start(cosT[:], cosT_d[:, :])
        nc.sync.dma_start(sinT[:], sinT_d[:, :])
        for q in range(1, 4):
            nc.sync.dma_start(hidt[:, q * w:(q + 1) * w],
                              hid_d[:, q * w:(q + 1) * w])
        nc.sync.dma_start(wvts[1][:], wv_d[:, 4 * KVD:8 * KVD])

        # ---- v projection first (1-pass fp16) so the AG can fire early ----
        pv = [ps.tile([P, ROWS], f32, tag="ps", name=f"pv{g}") for g in range(KV)]
        for q in range(4):
            wvt = wvts[q]
            for kq in range(4):
                kk = q * 4 + kq
                rhs = hidt[:, kk * CHUNK:(kk + 1) * CHUNK]
                for g in range(KV):
                    nc.tensor.matmul(pv[g][:],
                                     wvt[:, kq * KVD + g * P:kq * KVD + (g + 1) * P],
                                     rhs, start=(kk == 0), stop=(kk == KT - 1))
            if q + 2 < 4:
                # emit late so a busy slot never blocks the DMA FIFO
                nc.sync.dma_start(wvts[q + 2][:],
                                  wv_d[:, (q + 2) * 4 * KVD:(q + 3) * 4 * KVD])

        # RoPE on v (f32 DVE) then round to fp16; transpose to row-major
        vrow = [sb.tile([P, KVD], f16, tag="vrow", bufs=4, name=f"vrow{rt}")
                for rt in range(SLOTS)]
        for g in range(KV):
            vr = sb.tile([P, ROWS], f32, tag="vr", bufs=1)
            nc.scalar.copy(vr[:], pv[g][:])
            rot = sb.tile([P, ROWS], f32, tag="rot", bufs=1)
            nc.vector.tensor_scalar_mul(rot[0:64, :], vr[64:128, :], -1.0)
            nc.vector.tensor_copy(rot[64:128, :], vr[0:64, :])
            nc.vector.tensor_mul(vr[:], vr[:], cosT[:])
            nc.vector.tensor_mul(rot[:], rot[:], sinT[:])
            vr16 = sb.tile([P, ROWS], f16, tag="vr16", bufs=1)
            nc.vector.tensor_add(vr16[:], vr[:], rot[:])
            for rt in range(SLOTS):
                tp = ps.tile([P, P], f16, tag="ps", name=f"tpv{g}_{rt}")
                nc.tensor.transpose(tp[:], vr16[:, rt * P:(rt + 1) * P], id16[:])
                if rt % 2:
                    nc.scalar.copy(vrow[rt][:, g * P:(g + 1) * P], tp[:])
                else:
                    nc.vector.tensor_copy(vrow[rt][:, g * P:(g + 1) * P], tp[:])

        # ---- ONE AllGather of row-major v (within 4-core batch group) ----
        vr_in = dram.tile([ROWS, KVD], f16)
        vr_all = dram.tile([RANKS * ROWS, KVD], f16)
        for rt in range(SLOTS):
            nc.gpsimd.dma_start(vr_in[rt * P:(rt + 1) * P, :], vrow[rt][:])
        nc.gpsimd.collective_compute(
            "AllGather", mybir.AluOpType.bypass, replica_groups=rg,
            ins=[vr_in.opt()], outs=[vr_all.opt()])

        # ---- k projection (1-pass fp16) while the AG runs ----
        pk = [ps.tile([P, ROWS], f32, tag="ps", name=f"pk{g}") for g in range(KV)]
        for q in range(4):
            wkt = sb.tile([P, 4 * KVD], f16, tag="wk", bufs=2, name=f"wk{q}")
            nc.scalar.dma_start(wkt[:], wk_d[:, q * 4 * KVD:(q + 1) * 4 * KVD])
            for kq in range(4):
                kk = q * 4 + kq
                rhs = hidt[:, kk * CHUNK:(kk + 1) * CHUNK]
                for g in range(KV):
                    nc.tensor.matmul(pk[g][:],
                                     wkt[:, kq * KVD + g * P:kq * KVD + (g + 1) * P],
                                     rhs, start=(kk == 0), stop=(kk == KT - 1))
        kT16 = []
        for g in range(KV):
            t = sb.tile([P, ROWS], f16, tag="kT", bufs=4, name=f"kT{g}")
            nc.scalar.mul(t[:], pk[g][:], SCALING)
            kT16.append(t)

        # ---- gate slab helper (fp16 matmul; sigmoid deferred+batched) ----
        # explicit interleaving: engine streams are static, so gate slabs are
        # emitted exactly where their dense PE work fills a stall (the AG
        # wait, each head's softmax bubbles).  The sigmoid runs later as ONE
        # batched in-place pass - interleaving Sigmoid with the softmax's Exp
        # makes the scalar engine reload its function table (1.3us) per
        # switch.
        sigT = [None] * 16

        def gate_slab(nb):
            wq = sb.tile([P, KT * P], f16, tag="wqs", bufs=2, name=f"wq{nb}")
            nc.sync.dma_start(wq[:], wqg_d[nb, :, :])
            pg = ps.tile([P, ROWS], f32, tag="ps", name=f"pg{nb}")
            for kk in range(KT):
                nc.tensor.matmul(pg[:], wq[:, kk * P:(kk + 1) * P],
                                 hidt[:, kk * CHUNK:(kk + 1) * CHUNK],
                                 start=(kk == 0), stop=(kk == KT - 1))
            t = sb.tile([P, ROWS], f16, tag="sigT", bufs=16, name=f"sig{nb}")
            nc.vector.tensor_copy(t[:], pg[:])
            sigT[nb] = t

        for nb in range(9):
            gate_slab(nb)

        # ---- gathered loads (contiguous [128, 512] tiles) ----
        # vr_all row-block i = r*SLOTS + rt holds global t-block bi = 4*rt + r.
        vrg = []
        for i in range(RANKS * SLOTS):
            t = sb.tile([P, KVD], f16, tag="vrg", bufs=16, name=f"vrg{i}")
            if i % 2:
                nc.scalar.dma_start(t[:], vr_all[i * P:(i + 1) * P, :])
            else:
                nc.gpsimd.dma_start(t[:], vr_all[i * P:(i + 1) * P, :])
            vrg.append(t)

        # rebuild d-major score-side tiles with PE transposes
        # vtc[g*NCHUNK+c]: [128 d(g), 512 t], t-block 4c+r from vrg[r*SLOTS+c]
        vtc = [sb.tile([P, CHUNK], f16, tag="vtc", bufs=16, name=f"vtc{i}")
               for i in range(KV * NCHUNK)]
        for c in range(NCHUNK):
            for r in range(RANKS):
                src = vrg[r * SLOTS + c]
                for g in range(KV):
                    tp = ps.tile([P, P], f16, tag="ps", name=f"tpc{c}_{r}_{g}")
                    nc.tensor.transpose(tp[:], src[:, g * P:(g + 1) * P], id16[:])
                    dst = vtc[g * NCHUNK + c][:, r * P:(r + 1) * P]
                    nc.vector.tensor_copy(dst, tp[:])

        wos = [None] * 4

        # ---- attention per kv head, gate slabs filling the bubbles ----
        avT = [None] * KV
        gat = [None] * (KV * G)
        mts = {}
        for g in range(KV):
            # attnT_big[t(128), bi(16), s(512)]: normalized probs, [t,s] layout
            attnTB = sb.tile([P, NB, ROWS], f16, tag="attnTB", bufs=2,
                             name=f"attnTB{g}")

            def emit_tpa(k, comp, attn, dg):
                # transpose+normalize attn -> attnTB columns (bi, slot k);
                # deferred one slot so the PE stream never head-of-line
                # blocks on this slot's softmax chain.
                for ci, c in enumerate(comp):
                    tp = ps.tile([P, 4, P], f32, tag="ps", name=f"tpa{k}_{ci}")
                    for i in range(4):
                        nc.tensor.matmul(
                            tp[:, i, :],
                            attn[:, ci * CHUNK + i * P:ci * CHUNK + (i + 1) * P],
                            dg[:], start=True, stop=True)
                    nc.vector.tensor_copy(
                        attnTB[:, 4 * c:4 * c + 4, k * P:(k + 1) * P], tp[:])

            pending = None
            for k in range(SLOTS):
                comp = computed[k]
                nchk = len(comp)
                attn = sb.tile([P, CHUNK * nchk], f16, tag="attn", bufs=2,
                               padded_shape=[P, CHUNK * NCHUNK],
                               name=f"attn{g}_{k}")
                # two-phase softmax: all chunk matmuls + maxes run in
                # parallel, then one exp wave with the final max.
                pscs = []
                cms = []
                for ci, c in enumerate(comp):
                    psc = ps.tile([P, CHUNK], f32, tag="ps", name=f"psc{ci}")
                    nc.tensor.matmul(psc[:], kT16[g][:, k * P:(k + 1) * P],
                                     vtc[g * NCHUNK + c][:], start=True, stop=True)
                    if classes[k][c] == 2:
                        ai = add_idx[(k, c)]
                        if ai not in mts:
                            mt = sb.tile([P, CHUNK], bf16, tag="msk", bufs=4,
                                         name=f"mt{ai}")
                            nc.gpsimd.dma_start(
                                mt[:], mask_d[:, ai * CHUNK:(ai + 1) * CHUNK])
                            mts[ai] = mt
                        nc.vector.tensor_add(psc[:], psc[:], mts[ai][:])
                    cm = sb.tile([P, 1], f32, tag="stat", bufs=16, name=f"cm{ci}")
                    nc.vector.tensor_reduce(cm[:], psc[:], mybir.AxisListType.X,
                                            Alu.max, negate=True)
                    pscs.append(psc)
                    cms.append(cm)
                mneg = cms[0]   # -max
                for ci in range(1, nchk):
                    mnew = sb.tile([P, 1], f32, tag="stat", bufs=16, name=f"mn{ci}")
                    nc.vector.tensor_tensor(mnew[:], mneg[:], cms[ci][:], Alu.min)
                    mneg = mnew
                tot = None
                for ci in range(nchk):
                    csum = sb.tile([P, 1], f32, tag="stat", bufs=16, name=f"cs{ci}")
                    nc.scalar.activation(attn[:, ci * CHUNK:(ci + 1) * CHUNK],
                                         pscs[ci][:], Act.Exp, bias=mneg[:],
                                         accum_out=csum[:])
                    if tot is None:
                        tot = csum
                    else:
                        t2 = sb.tile([P, 1], f32, tag="stat", bufs=16, name=f"tt{ci}")
                        nc.vector.tensor_add(t2[:], tot[:], csum[:])
                        tot = t2
                rinv = sb.tile([P, 1], f32, tag="stat", bufs=16)
                nc.vector.reciprocal(rinv[:], tot[:])
                # diag(1/sum): the transpose matmul applies the softmax
                # normalization for free.
                dg = sb.tile([P, P], f16, tag="dg", bufs=2, name=f"dg{g}_{k}")
                nc.vector.tensor_scalar_mul(dg[:], id16[:], rinv[:])
                if k % 2 and 9 + 2 * g + k // 2 < 16:
                    gate_slab(9 + 2 * g + k // 2)
                if pending is not None:
                    emit_tpa(*pending)
                pending = (k, comp, attn, dg)
            emit_tpa(*pending)
            # attn @ v  ->  avT[g] [128 d, 512 s]
            todo = []
            for bi in range(NB):
                ks = [k for k in range(SLOTS) if (bi // RANKS) in computed[k]]
                if ks:
                    todo.append((bi, ks[0]))
            pav = ps.tile([P, ROWS], f32, tag="ps")
            for n, (bi, kmin) in enumerate(todo):
                lhs = vrg[(bi % RANKS) * SLOTS + (bi // RANKS)][:, g * P:(g + 1) * P]
                nc.tensor.matmul(pav[:, kmin * P:ROWS], lhs,
                                 attnTB[:, bi, kmin * P:ROWS],
                                 start=(n == 0), stop=(n == len(todo) - 1))
            t = sb.tile([P, ROWS], f16, tag="avT", bufs=4, name=f"avT{g}")
            nc.vector.tensor_copy(t[:], pav[:])
            avT[g] = t
            if g == 1:
                wos[0] = sb.tile([P, KT * CHUNK], f16, tag="wos", bufs=2,
                                 name="wo0")
                nc.sync.dma_start(wos[0][:], wo_d[0, :, :])
        # batched sigmoid pass, in place.  The zero "token" bias is written
        # only after the last head: a fake data dependency that stops the
        # scheduler from hoisting sigmoids between the softmax Exps (every
        # Sigmoid<->Exp switch reloads the ACT function table, 1.3us).
        token = sb.tile([P, 1], f32, tag="stat", bufs=16)
        nc.vector.tensor_scalar_mul(token[:], avT[KV - 1][:, 0:1], 0.0)
        for nb in range(16):
            nc.scalar.activation(sigT[nb][:], sigT[nb][:], Act.Sigmoid,
                                 bias=token[:])

        # ---- gated = tile_G(avT) * sigT  (fp16) ----
        for g in range(KV):
            for i in range(G):
                t = sb.tile([P, ROWS], f16, tag="gat", bufs=16,
                            name=f"gat{g}_{i}")
                nc.vector.tensor_mul(t[:], avT[g][:], sigT[4 * g + i][:])
                gat[4 * g + i] = t

        # ---- out projection (fp16) ----
        for nblk in range(4):
            if wos[nblk] is None:
                wos[nblk] = sb.tile([P, KT * CHUNK], f16, tag="wos", bufs=2,
                                    name=f"wo{nblk}")
                nc.sync.dma_start(wos[nblk][:], wo_d[nblk, :, :])
            wob = wos[nblk]
            for rt in range(SLOTS):
                po = ps.tile([P, CHUNK], f32, tag="ps")
                for cc in range(KT):
                    nc.tensor.matmul(po[:], gat[cc][:, rt * P:(rt + 1) * P],
                                     wob[:, cc * CHUNK:(cc + 1) * CHUNK],
                                     start=(cc == 0), stop=(cc == KT - 1))
                t = sb.tile([P, CHUNK], bf16, tag="oev", bufs=2,
                            name=f"oev{nblk}_{rt}")
                if rt % 2:
                    nc.vector.tensor_copy(t[:], po[:])
                else:
                    nc.scalar.copy(t[:], po[:])
                nc.sync.dma_start(
                    out_d[rt * P:(rt + 1) * P, nblk * CHUNK:(nblk + 1) * CHUNK], t[:])
            if nblk + 1 < 4 and wos[nblk + 1] is None:
                wos[nblk + 1] = sb.tile([P, KT * CHUNK], f16, tag="wos", bufs=2,
                                        name=f"wo{nblk + 1}")
                nc.sync.dma_start(wos[nblk + 1][:], wo_d[nblk + 1, :, :])

    nc.compile()
    return nc


def _pack_kt(w):
    """[HS, N] -> [128, KT*N] with row p carrying all kk-tiles contiguously."""
    n = w.shape[1]
    return np.ascontiguousarray(
        w.reshape(KT, P, n).transpose(1, 0, 2).reshape(P, KT * n))


def kernel(hidden_states, cos, sin, attention_mask, Wq, Wk, Wv, Wo):
    import ml_dtypes
    from concourse.bass_utils import run_bass_kernel_spmd

    hidden_states = np.asarray(hidden_states, dtype=np.float32)
    cos = np.asarray(cos, dtype=np.float32)
    sin = np.asarray(sin, dtype=np.float32)
    mask = np.asarray(attention_mask, dtype=np.float32)[0, 0]
    Wq = np.asarray(Wq, dtype=np.float32)
    Wk = np.asarray(Wk, dtype=np.float32)
    Wv = np.asarray(Wv, dtype=np.float32)
    Wo = np.asarray(Wo, dtype=np.float32)

    classes = _mask_classes(mask)
    key = tuple(tuple(r) for r in classes)
    if key not in _CACHE:
        _CACHE[key] = _build(classes)
    nc = _CACHE[key]

    # shared (identical across cores) staged weights, packed + fp16
    wv16 = _pack_kt(Wv).astype(np.float16)
    wk16 = _pack_kt(Wk).astype(np.float16)
    wqg16 = np.ascontiguousarray(
        Wq[:, HS:].reshape(KT, P, 16, P).transpose(2, 1, 0, 3)
        .reshape(16, P, KT * P)).astype(np.float16)
    wo16 = np.ascontiguousarray(
        Wo.reshape(KT, P, 4, CHUNK).transpose(2, 1, 0, 3)
        .reshape(4, P, KT * CHUNK)).astype(np.float16)

    in_maps = []
    for core in range(NCORES):
        b, j = divmod(core, RANKS)
        blocks = [RANKS * k + j for k in range(SLOTS)]
        rows = np.concatenate([np.arange(bi * P, (bi + 1) * P) for bi in blocks])
        strips = []
        for k in range(SLOTS):
            for c in range(NCHUNK):
                if classes[k][c] == 2:
                    bi = RANKS * k + j
                    strips.append(mask[bi * P:(bi + 1) * P,
                                       c * CHUNK:(c + 1) * CHUNK])
        if not strips:
            strips.append(np.zeros((P, CHUNK), np.float32))
        maskst = np.concatenate(strips, axis=1).astype(ml_dtypes.bfloat16)
        hid16 = _pack_kt(np.ascontiguousarray(
            hidden_states[b][rows].T)).astype(np.float16)
        in_maps.append({
            "hid16": hid16,
            "wv16": wv16,
            "wk16": wk16,
            "wqg16": wqg16,
            "wo16": wo16,
            "cosT": np.ascontiguousarray(cos[b][rows].T).astype(np.float16),
            "sinT": np.ascontiguousarray(sin[b][rows].T).astype(np.float16),
            "maskst": np.ascontiguousarray(maskst),
        })

    res = run_bass_kernel_spmd(nc, in_maps, core_ids=list(range(NCORES)))

    out = np.empty((B, S, HS), np.float32)
    for core in range(NCORES):
        b, j = divmod(core, RANKS)
        o = np.asarray(res.results[core]["out"]).astype(np.float32)
        for k in range(SLOTS):
            bi = RANKS * k + j
            out[b, bi * P:(bi + 1) * P, :] = o[k * P:(k + 1) * P, :]
    return out
